# revision 1
# baseline (speedup 1.0000x reference)
"""DINOv3 ViT attention (RoPE + det-temp scaling + additive gate) on 8 TRN2 cores.

Sharding: pure data-parallel over batch (B=8 -> 1 batch element per core).
Weights / gate / rope tables replicated. No collectives.

Per-core algorithm (all matmuls fp32r = fp32 rounded to 11 mantissa bits,
1 PE cycle/column; S padded 1129->1152):
  phase 1: qT/kT [dout,s] and v [s,dout] projections from host-transposed
           hsT/weights; v bias via a K=1 ones-row matmul; q bias + 1/sqrt(hd)
           folded into the ACT eviction.
  phase 2: RoPE: rotate_half via a PE permutation matmul, then 3 DVE ops
           against stacked cos/sin tables; det-temp scaling via a PSUM
           outer-product pattern tile; fused per dout-tile, in place.
  phase 3: scoresT[sk,sq] = gate^T (identity-matmul copy into PSUM) +
           kT^T q (K=64, head pairs on disjoint 64-row PE strips via
           tile_position so they run concurrently); exp on ACT (no max
           subtraction -- scores are O(1) by construction); ctxT[hd+1,sq]
           accumulated over sk with a ones column in v producing the softmax
           denominator; normalization = DVE reciprocal of the denominator row
           + PE outer-product broadcast + DVE multiply; output projection
           accumulated over ctxT tiles + o_b added at eviction.

The harness contract: kernel(**inputs) with FULL inputs, returns FULL output.
"""
import numpy as np
from contextlib import ExitStack

import concourse.bacc as bacc
import concourse.mybir as mybir
import concourse.tile as tile
from concourse.bass_utils import run_bass_kernel_spmd

F32 = mybir.dt.float32
F32R = mybir.dt.float32r
AF = mybir.ActivationFunctionType

# ---------------- problem config (hardcoded per harness contract) ------------


class CFG:
    B = 8
    S = 1129
    SP = 1152            # padded S (9 * 128)
    D = 768
    H = 12
    HD = 64
    ROPE_START = 5
    ROPE_END = 1029
    DET_START = 1029
    DET_END = 1129
    P_SCALE = 2.0
    N_CORES = 8
    SQB = 384            # sq block (>=256 keeps fp32r at 1 cyc/row; 3 | SP)
    GATE_NEG = -30.0     # gate value for pad keys: exp(-30) ~ 9e-14
    ONLY_PHASE1 = False

    @property
    def KT(self):
        return self.D // 128          # dout/din 128-tiles (6)

    @property
    def NT(self):
        return self.SP // 128         # s 128-tiles (9)

    @property
    def NB(self):
        return self.SP // self.SQB    # sq blocks (3)

    @property
    def ROPE_LEN(self):
        return self.ROPE_END - self.ROPE_START


def round_f32r(x: np.ndarray) -> np.ndarray:
    """Round fp32 to the fp32r format (11 mantissa bits, RNE)."""
    b = np.ascontiguousarray(x, dtype=np.float32).view(np.uint32)
    low = b & np.uint32(0xFFF)
    b = b & np.uint32(0xFFFFF000)
    rnd = (low > 0x800) | ((low == 0x800) & (((b >> 12) & 1) != 0))
    b = b + (rnd.astype(np.uint32) << 12)
    return b.view(np.float32)


# ---------------- device program ------------------------------------------


def build_nc(cfg: CFG):
    nc = bacc.Bacc(trn_type="TRN2", target_bir_lowering=False, debug=False)
    KT, NT, NB, SQB, SP = cfg.KT, cfg.NT, cfg.NB, cfg.SQB, cfg.SP
    H, HD = cfg.H, cfg.HD
    RS, RE, DS, DE = cfg.ROPE_START, cfg.ROPE_END, cfg.DET_START, cfg.DET_END
    RL = cfg.ROPE_LEN
    DET = DE - DS

    # ---- dram parameters (per core) ----
    d_hsT = nc.dram_tensor("hsT", [cfg.D, SP], F32R, kind="ExternalInput").ap()
    d_qwT = nc.dram_tensor("qwT", [cfg.D, cfg.D], F32R, kind="ExternalInput").ap()
    d_kwT = nc.dram_tensor("kwT", [cfg.D, cfg.D], F32R, kind="ExternalInput").ap()
    d_vwT = nc.dram_tensor("vwT", [(KT + 1) * 128, cfg.D], F32R, kind="ExternalInput").ap()
    d_owT = nc.dram_tensor("owT", [cfg.D, cfg.D], F32R, kind="ExternalInput").ap()
    d_gateT = nc.dram_tensor("gateT", [SP, SP], F32R, kind="ExternalInput").ap()
    d_qb = nc.dram_tensor("qb", [128, KT], F32, kind="ExternalInput").ap()
    d_ob = nc.dram_tensor("ob", [128, cfg.D], F32, kind="ExternalInput").ap()
    d_cosT2 = nc.dram_tensor("cosT2", [128, RL], F32, kind="ExternalInput").ap()
    d_sinT2 = nc.dram_tensor("sinT2", [128, RL], F32, kind="ExternalInput").ap()
    d_rotT = nc.dram_tensor("rotT", [128, 128], F32R, kind="ExternalInput").ap()
    d_ident = nc.dram_tensor("ident", [128, 128], F32R, kind="ExternalInput").ap()
    d_ones65 = nc.dram_tensor("ones65", [65, 128], F32R, kind="ExternalInput").ap()
    d_onescol = nc.dram_tensor("onescol", [128, H], F32R, kind="ExternalInput").ap()
    d_masks = nc.dram_tensor("masks", [1, 256], F32R, kind="ExternalInput").ap()
    d_ph = nc.dram_tensor("ph", [1, DET], F32, kind="ExternalInput").ap()
    d_pw = nc.dram_tensor("pw", [1, DET], F32, kind="ExternalInput").ap()
    d_out = nc.dram_tensor("out", [SP, cfg.D], F32, kind="ExternalOutput").ap()

    with tile.TileContext(nc) as tc, ExitStack() as gctx:
        # ---------------- global pools (span the whole kernel) --------------
        gsb = gctx.enter_context(tc.tile_pool(name="gsb", bufs=1))

        # small constants
        t_ident = gsb.tile([128, 128], F32R, tag="ident")
        nc.sync.dma_start(t_ident[:], d_ident[:, :])
        t_ones65 = gsb.tile([65, 128], F32R, tag="ones65")
        nc.sync.dma_start(t_ones65[:], d_ones65[:, :])


        # gate tiles live in the global pool; DMAs are emitted later (after the
        # phase-1 critical loads) so they don't delay hsT/weights at startup
        t_gate = [gsb.tile([128, SP], F32R, tag=f"g{t}", name=f"g{t}")
                  for t in range(NT)]

        # persistent activation tensors
        t_qTf = [gsb.tile([128, SP], F32R, tag=f"qTf{m}", name=f"qTf{m}") for m in range(KT)]
        t_kTf = [gsb.tile([128, SP], F32R, tag=f"kTf{m}", name=f"kTf{m}") for m in range(KT)]
        t_v = [gsb.tile([128, H * 65], F32R, tag=f"v{t}", name=f"v{t}") for t in range(NT)]

        # ==================== phase 1 + 2: projections & q/k finalize =======
        with ExitStack() as p1:
            sb1 = p1.enter_context(tc.tile_pool(name="sb1", bufs=1))
            wsb = p1.enter_context(tc.tile_pool(name="wsb", bufs=2))
            tsb = p1.enter_context(tc.tile_pool(name="tsb", bufs=1))
            ps_q = p1.enter_context(tc.tile_pool(name="ps_q", bufs=2, space="PSUM"))
            ps_big = p1.enter_context(tc.tile_pool(name="ps_big", bufs=2, space="PSUM"))
            ps_tem = p1.enter_context(tc.tile_pool(name="ps_tem", bufs=1, space="PSUM"))

            # hsT tiles interleaved with qw loads so the first q chain can
            # start accumulating as tiles arrive
            t_hsT = []
            qw = []
            for k in range(KT):
                t = sb1.tile([128, SP], F32R, tag=f"hsT{k}", name=f"hsT{k}")
                eng = nc.sync if k % 2 == 0 else nc.scalar
                eng.dma_start(t[:], d_hsT[k * 128:(k + 1) * 128, :])
                t_hsT.append(t)
                w = wsb.tile([128, cfg.D], F32R, tag=f"w{k}", name=f"qw{k}")
                weng = nc.scalar if k % 2 == 0 else nc.sync
                weng.dma_start(w[:], d_qwT[k * 128:(k + 1) * 128, :])
                qw.append(w)


            # rope/det tables
            t_cos = sb1.tile([128, RL], F32, tag="cos")
            nc.scalar.dma_start(t_cos[:], d_cosT2[:, :])
            t_sin = sb1.tile([128, RL], F32, tag="sin")
            nc.scalar.dma_start(t_sin[:], d_sinT2[:, :])
            t_rotT = sb1.tile([128, 128], F32R, tag="rotT")
            nc.sync.dma_start(t_rotT[:], d_rotT[:, :])
            t_qb = sb1.tile([128, KT], F32, tag="qb")
            nc.sync.dma_start(t_qb[:], d_qb[:, :])

            # det temperature pattern tile: [128, DET] via two outer products
            t_ms = sb1.tile([1, 256], F32R, tag="ms")
            nc.sync.dma_start(t_ms[:], d_masks[:, :])
            t_ph = sb1.tile([1, DET], F32, tag="ph")
            nc.sync.dma_start(t_ph[:], d_ph[:, :])
            t_pw = sb1.tile([1, DET], F32, tag="pw")
            nc.sync.dma_start(t_pw[:], d_pw[:, :])
            t_eh = sb1.tile([1, DET], F32R, tag="eh")
            nc.scalar.activation(t_eh[:], t_ph[:], AF.Exp, bias=0.0, scale=cfg.P_SCALE)
            t_ew = sb1.tile([1, DET], F32R, tag="ew")
            nc.scalar.activation(t_ew[:], t_pw[:], AF.Exp, bias=0.0, scale=cfg.P_SCALE)
            p_tem = ps_tem.tile([128, DET], F32, tag="tem")
            nc.tensor.matmul(p_tem[:], t_ms[0:1, 0:128], t_eh[:], start=True, stop=False)
            nc.tensor.matmul(p_tem[:], t_ms[0:1, 128:256], t_ew[:], start=False, stop=True)

            def load_w(dram, k):
                t = wsb.tile([128, cfg.D], F32R, tag=f"w{k}")
                nc.sync.dma_start(t[:], dram[k * 128:(k + 1) * 128, :])
                return t

            def finalize_qk(raw, dst):
                """RoPE + det-temp + prefix/tail copy: raw [128,SP] F32 -> dst F32R."""
                p_rot = ps_big.tile([128, RL], F32, tag="big")
                for c0 in range(0, RL, 512):
                    cw = min(512, RL - c0)
                    nc.tensor.matmul(p_rot[:, c0:c0 + cw], t_rotT[:],
                                     raw[:, RS + c0:RS + c0 + cw],
                                     start=True, stop=True)
                tmp1 = tsb.tile([128, RL], F32, tag="tmp1")
                nc.vector.tensor_mul(tmp1[:], p_rot[:], t_sin[:])
                # in-place: dst *= cos (WAR vs the rot-matmul read), then += tmp1
                # (on GpSimd: SBUF-only operands, frees the DVE for the psum ops)
                nc.gpsimd.tensor_mul(dst[:, RS:RE], raw[:, RS:RE], t_cos[:])
                nc.vector.tensor_add(dst[:, RS:RE], dst[:, RS:RE], tmp1[:])
                # det region: multiply by the temperature pattern (psum operand)
                nc.vector.tensor_mul(dst[:, DS:DE], raw[:, DS:DE], p_tem[:])
                if raw is not dst:
                    nc.vector.tensor_copy(dst[:, 0:RS], raw[:, 0:RS])
                    if SP > DE:
                        nc.vector.tensor_copy(dst[:, DE:SP], raw[:, DE:SP])

            # ---- qT: out[dout_tile, s] ----
            for m in range(KT):
                for nb0 in range(0, SP, SQB):
                    p = ps_q.tile([128, SQB], F32, tag="qp")
                    for k in range(KT):
                        nc.tensor.matmul(p[:], qw[k][:, m * 128:(m + 1) * 128],
                                         t_hsT[k][:, nb0:nb0 + SQB],
                                         start=(k == 0), stop=(k == KT - 1))
                    nc.scalar.activation(t_qTf[m][:, nb0:nb0 + SQB], p[:], AF.Identity,
                                         bias=t_qb[:, m:m + 1], scale=cfg.HD ** -0.5)
                if m > 0:
                    finalize_qk(t_qTf[m - 1], t_qTf[m - 1])

            # ---- kT ----
            kw = [load_w(d_kwT, k) for k in range(KT)]
            # gate DMAs: after the k weights, well before attention needs them
            for t in range(NT):
                nc.sync.dma_start(t_gate[t][:], d_gateT[t * 128:(t + 1) * 128, :])
            for m in range(KT):
                for nb0 in range(0, SP, SQB):
                    p = ps_q.tile([128, SQB], F32, tag="qp", name="kp")
                    for k in range(KT):
                        nc.tensor.matmul(p[:], kw[k][:, m * 128:(m + 1) * 128],
                                         t_hsT[k][:, nb0:nb0 + SQB],
                                         start=(k == 0), stop=(k == KT - 1))
                    nc.scalar.copy(t_kTf[m][:, nb0:nb0 + SQB], p[:])
                if m == 0:
                    finalize_qk(t_qTf[KT - 1], t_qTf[KT - 1])
                if m > 0:
                    finalize_qk(t_kTf[m - 1], t_kTf[m - 1])
            finalize_qk(t_kTf[KT - 1], t_kTf[KT - 1])

            # ---- v: out[s_tile, dout] interleaved with a ones column per head
            vw = [load_w(d_vwT, k) for k in range(KT)]
            t_vb = sb1.tile([1, cfg.D], F32R, tag="vb")
            nc.sync.dma_start(t_vb[:], d_vwT[cfg.D:cfg.D + 1, :])
            t_onescol = sb1.tile([128, H], F32R, tag="onescol")
            nc.sync.dma_start(t_onescol[:], d_onescol[:, :])
            for mt in range(NT):
                p = ps_big.tile([128, cfg.D], F32, tag="big")
                for n0 in range(0, cfg.D, 512):
                    nw = min(512, cfg.D - n0)
                    nc.tensor.matmul(p[:, n0:n0 + nw],
                                     t_ones65[0:1, :],
                                     t_vb[0:1, n0:n0 + nw],
                                     start=True, stop=False)
                    for k in range(KT):
                        nc.tensor.matmul(p[:, n0:n0 + nw],
                                         t_hsT[k][:, mt * 128:(mt + 1) * 128],
                                         vw[k][:, n0:n0 + nw],
                                         start=False, stop=(k == KT - 1))
                vin = p[:, :].rearrange("p (h j) -> p h j", h=H)
                v3 = t_v[mt][:, :].rearrange("p (h j) -> p h j", j=65)
                nc.scalar.activation(v3[:, :, 0:HD], vin, AF.Identity,
                                     bias=0.0, scale=1.0)
                oc3 = t_onescol[:, :].rearrange("p (h o) -> p h o", o=1)
                nc.vector.tensor_copy(v3[:, :, HD:65], oc3)

        # ==================== phase 3: attention + output projection ========
        with ExitStack() as p3:
          if not cfg.ONLY_PHASE1:
              sb3 = p3.enter_context(tc.tile_pool(name="sb3", bufs=1))
              esb = p3.enter_context(tc.tile_pool(name="esb", bufs=8))
              csb = p3.enter_context(tc.tile_pool(name="csb", bufs=2))
              ps_sc = p3.enter_context(tc.tile_pool(name="ps_sc", bufs=4, space="PSUM"))
              ps_ctx = p3.enter_context(tc.tile_pool(name="ps_ctx", bufs=4, space="PSUM"))

              t_ob = sb3.tile([128, cfg.D], F32, tag="ob")
              nc.sync.dma_start(t_ob[:], d_ob[:, :])
              # output-projection weights
              t_ow = []
              for k in range(KT):
                  w = sb3.tile([128, cfg.D], F32R, tag=f"ow{k}")
                  nc.sync.dma_start(w[:], d_owT[k * 128:(k + 1) * 128, :])
                  t_ow.append(w)

              def norm_head(h, p_ctx, t_ctxT):
                  """1/den broadcast over 64 hd rows, write into the ctxT tile."""
                  ht, hr = h // 2, (h % 2) * 64
                  t_rc = csb.tile([65, SQB], F32R, tag="recip", bufs=3, name="rc")
                  with nc.allow_low_precision(reason="f32r recip of softmax denom"):
                      nc.vector.reciprocal(t_rc[64:65, :], p_ctx[64:65, :])
                  p_bc = ps_ctx.tile([64, SQB], F32, tag="ctx", name="bc")
                  nc.tensor.matmul(p_bc[:], t_ones65[64:65, 0:64], t_rc[64:65, :],
                                   start=True, stop=True)
                  t_cu = csb.tile([64, SQB], F32, tag="cu", bufs=3, name="cu")
                  nc.vector.tensor_copy(t_cu[:], p_ctx[0:64, :])
                  if hr == 0:
                      nc.vector.tensor_mul(t_ctxT[ht][0:64, :], t_cu[:], p_bc[:])
                  else:
                      t_hc = csb.tile([64, SQB], F32R, tag="hctx", bufs=3, name="hc")
                      nc.vector.tensor_mul(t_hc[:], t_cu[:], p_bc[:])
                      nc.sync.dma_start(t_ctxT[ht][64:128, :], t_hc[:])

              for b in range(NB):
                  b0 = b * SQB
                  t_ctxT = [csb.tile([128, SQB], F32R, tag=f"ctxT{k}", name=f"ctxT{k}", bufs=3)
                            for k in range(KT)]
                  # heads processed in pairs: the two K=64 score matmuls target
                  # disjoint 64-row strips of the PE array (row tiling) and run
                  # concurrently on hardware
                  for hp in range(H // 2):
                      h0, h1 = 2 * hp, 2 * hp + 1
                      p_ctx0 = ps_ctx.tile([65, SQB], F32, tag="ctx", name="ctx0")
                      p_ctx1 = ps_ctx.tile([65, SQB], F32, tag="ctx", name="ctx1")
                      for skt in range(NT):
                          sc0 = ps_sc.tile([128, 512], F32, tag="sc", name="sc0")
                          sc1 = ps_sc.tile([128, 512], F32, tag="sc", name="sc1")
                          nc.tensor.matmul(sc0[:, 0:SQB], t_ident[:],
                                           t_gate[skt][:, b0:b0 + SQB],
                                           start=True, stop=False)
                          nc.tensor.matmul(sc1[:, 0:SQB], t_ident[:],
                                           t_gate[skt][:, b0:b0 + SQB],
                                           start=True, stop=False)
                          nc.tensor.matmul(sc0[:, 0:SQB],
                                           t_kTf[hp][0:64, skt * 128:(skt + 1) * 128],
                                           t_qTf[hp][0:64, b0:b0 + SQB],
                                           start=False, stop=True,
                                           tile_position=(0, 0))
                          nc.tensor.matmul(sc1[:, 0:SQB],
                                           t_kTf[hp][64:128, skt * 128:(skt + 1) * 128],
                                           t_qTf[hp][64:128, b0:b0 + SQB],
                                           start=False, stop=True,
                                           tile_position=(64, 0))
                          e0 = esb.tile([128, SQB], F32R, tag="exp", name="e0")
                          nc.scalar.activation(e0[:], sc0[:, 0:SQB], AF.Exp,
                                               bias=0.0, scale=1.0)
                          e1 = esb.tile([128, SQB], F32R, tag="exp", name="e1")
                          nc.scalar.activation(e1[:], sc1[:, 0:SQB], AF.Exp,
                                               bias=0.0, scale=1.0)
                          nc.tensor.matmul(p_ctx0[:], t_v[skt][:, h0 * 65:h0 * 65 + 65],
                                           e0[:], start=(skt == 0), stop=(skt == NT - 1))
                          nc.tensor.matmul(p_ctx1[:], t_v[skt][:, h1 * 65:h1 * 65 + 65],
                                           e1[:], start=(skt == 0), stop=(skt == NT - 1))
                      norm_head(h0, p_ctx0, t_ctxT)
                      norm_head(h1, p_ctx1, t_ctxT)

                  # output projection for this sq block (psum chunks share sc slots)
                  for mt in range(SQB // 128):
                      t_out = csb.tile([128, cfg.D], F32, tag="out")
                      for n0 in range(0, cfg.D, 512):
                          nw = min(512, cfg.D - n0)
                          p_o = ps_ctx.tile([128, nw], F32, tag="ctx", name="po")
                          for k in range(KT):
                              nc.tensor.matmul(p_o[:],
                                               t_ctxT[k][:, mt * 128:(mt + 1) * 128],
                                               t_ow[k][:, n0:n0 + nw],
                                               start=(k == 0), stop=(k == KT - 1))
                          nc.vector.tensor_add(t_out[:, n0:n0 + nw], p_o[:],
                                               t_ob[:, n0:n0 + nw])
                          r0 = b0 + mt * 128
                          nc.sync.dma_start(d_out[r0:r0 + 128, n0:n0 + nw],
                                            t_out[:, n0:n0 + nw])

    nc.compile()
    return nc


# ---------------- host-side prep + dispatch --------------------------------


def _host_prep(cfg: CFG, hidden_states, q_w, q_b, k_w, v_w, v_b, o_w, o_b,
               cos, sin, ph, pw, gate):
    KT, SP, H, HD = cfg.KT, cfg.SP, cfg.H, cfg.HD
    D, S = cfg.D, cfg.S
    DET = cfg.DET_END - cfg.DET_START
    half = HD // 2

    shared = {}
    shared["qwT"] = round_f32r(q_w.T)
    shared["kwT"] = round_f32r(k_w.T)
    vwT = np.zeros(((KT + 1) * 128, D), np.float32)
    vwT[:D] = v_w.T
    vwT[D] = v_b
    shared["vwT"] = round_f32r(vwT)
    shared["owT"] = round_f32r(o_w.T)
    gateT = np.zeros((SP, SP), np.float32)
    gateT[:S, :S] = gate[0, 0].T
    gateT[S:, :] = cfg.GATE_NEG
    shared["gateT"] = round_f32r(gateT)
    # biases: qb pre-scaled by 1/sqrt(hd), laid out [128, KT]
    qb = (q_b.astype(np.float32) * (HD ** -0.5)).reshape(KT, 128).T
    shared["qb"] = np.ascontiguousarray(qb)
    shared["ob"] = np.broadcast_to(o_b.astype(np.float32)[None, :], (128, D)).copy()
    # rope tables: [128, RL] = two stacked head-copies of cos/sin transposed
    cosT = cos.T.astype(np.float32)                       # [HD, RL]
    sinT = sin.T.astype(np.float32)
    shared["cosT2"] = np.vstack([cosT, cosT]).astype(np.float32)
    shared["sinT2"] = np.vstack([sinT, sinT]).astype(np.float32)
    # rotation matrix R (rotate_half along the hd partition dim), applied as
    # R @ x via lhsT = R.T; R spans two stacked heads per 128-partition tile
    R = np.zeros((128, 128), np.float32)
    for blk in range(2):
        o = blk * HD
        for j in range(half):
            R[o + j, o + half + j] = -1.0
            R[o + half + j, o + j] = 1.0
    shared["rotT"] = round_f32r(R.T)
    shared["ident"] = round_f32r(np.eye(128, dtype=np.float32))
    shared["ones65"] = round_f32r(np.ones((65, 128), np.float32))
    shared["onescol"] = round_f32r(np.ones((128, H), np.float32))
    maska = np.zeros((1, 128), np.float32)
    maskb = np.zeros((1, 128), np.float32)
    for p in range(128):
        if (p % HD) < half:
            maska[0, p] = 1.0
        else:
            maskb[0, p] = 1.0
    shared["masks"] = round_f32r(np.concatenate([maska, maskb], axis=1))
    shared["ph"] = ph.astype(np.float32).reshape(1, DET)
    shared["pw"] = pw.astype(np.float32).reshape(1, DET)

    in_maps = []
    for c in range(cfg.N_CORES):
        hsT = np.zeros((D, SP), np.float32)
        hsT[:, :S] = hidden_states[c].T
        m = dict(shared)
        m["hsT"] = round_f32r(hsT)
        in_maps.append(m)
    return in_maps


_NC_CACHE = {}


def kernel(hidden_states, q_w, q_b, k_w, v_w, v_b, o_w, o_b,
           cos, sin, ph, pw, gate,
           rope_start=5, rope_end=1029, det_start=1029, det_end=1129):
    cfg = CFG()
    in_maps = _host_prep(cfg, np.asarray(hidden_states, np.float32),
                         np.asarray(q_w, np.float32), np.asarray(q_b, np.float32),
                         np.asarray(k_w, np.float32), np.asarray(v_w, np.float32),
                         np.asarray(v_b, np.float32), np.asarray(o_w, np.float32),
                         np.asarray(o_b, np.float32), np.asarray(cos, np.float32),
                         np.asarray(sin, np.float32), np.asarray(ph, np.float32),
                         np.asarray(pw, np.float32), np.asarray(gate, np.float32))
    if "nc" not in _NC_CACHE:
        _NC_CACHE["nc"] = build_nc(cfg)
    nc = _NC_CACHE["nc"]
    res = run_bass_kernel_spmd(nc, in_maps, list(range(cfg.N_CORES)))
    out = np.stack([res.results[c]["out"][:cfg.S] for c in range(cfg.N_CORES)])
    return out.astype(np.float32)



# revision 4
# speedup vs baseline: 1.1848x; 1.1848x over previous
"""DINOv3 ViT attention (RoPE + det-temp scaling + additive gate) on 8 TRN2 cores.

Sharding: pure data-parallel over batch (B=8 -> 1 batch element per core).
Weights / gate / rope tables replicated. No collectives.

Per-core algorithm (fp8e4 DoubleRow matmuls: 2 K-tiles of 128 contracted per
pass at 0.5 PE cycles/column; S padded 1129->1152):
  phase 1: q/k/v projections from folded fp8 hsT/weight tiles (3 DoubleRow
           passes cover K=768); biases added as K=1 bf16 matmuls into PSUM;
           PSUM evicted by DVE tensor_scalar (scale folded) into f32r "raw"
           tiles.
  phase 2: RoPE on the raw tiles: rotate_half via an f32r PE permutation
           matmul; cos/sin tables pre-scaled by the fp8 quantization scale so
           the final DVE add writes fp8 q8/k8 score operands directly;
           det-temp via a PSUM outer-product pattern (also pre-scaled).
  phase 3: per (head-pair, key-strip): gate copied into PSUM with a folded-
           identity fp8 DoubleRow matmul (0.5 cyc/col); per-head score matmul
           as a DoubleRow with a zeroed second K-tile; ONE merged exp on ACT
           over both heads' PSUM banks -> fp8 e tiles laid out in key-strip
           pairs; attention*V as fp8 DoubleRow over strip pairs with a ones
           column in V producing the softmax denominator; normalization via
           DVE reciprocal + PE outer-product broadcast (broadcast constant
           carries the fp8 ctx scale); output projection fp8 DoubleRow with
           o-bias as a K=1 bf16 matmul and a single DVE descale eviction.

Scales (powers of two, folded into tables/biases/eviction constants):
  hs x16, proj weights x256, q8/k8 x16 (via cos/sin/tem tables),
  e x16 (exp bias ln16), v x16, ctx x64, o_w x256.
Pad keys are neutralized by zeroing V (and its denominator ones column) on
pad rows, so the pad gate value is irrelevant.

The harness contract: kernel(**inputs) with FULL inputs, returns FULL output.
"""
import math
import numpy as np
from contextlib import ExitStack

import ml_dtypes

import concourse.bacc as bacc
import concourse.mybir as mybir
import concourse.tile as tile
from concourse.bass_utils import run_bass_kernel_spmd

F32 = mybir.dt.float32
F32R = mybir.dt.float32r
F8 = mybir.dt.float8e4
BF16 = mybir.dt.bfloat16
AF = mybir.ActivationFunctionType
DR = mybir.MatmulPerfMode.DoubleRow

# ---------------- problem config (hardcoded per harness contract) ------------


class CFG:
    B = 8
    S = 1129
    SP = 1152            # padded S (9 * 128)
    D = 768
    H = 12
    HD = 64
    ROPE_START = 5
    ROPE_END = 1029
    DET_START = 1029
    DET_END = 1129
    P_SCALE = 2.0
    N_CORES = 8
    SQB = 384            # sq block (3 | SP/128 blocks of 128)

    # fp8 scale plan (all powers of two)
    S_H = 16.0           # hidden states
    S_W = 256.0          # q/k/v weights
    S_Q8 = 16.0          # q after rope (folded into cos/sin/tem tables)
    S_K8 = 16.0          # k after rope
    S_E = 16.0           # exp output (via exp bias ln(S_E))
    S_V = 16.0           # v
    S_C = 64.0           # normalized ctx
    S_OW = 256.0         # o weights

    @property
    def KT(self):
        return self.D // 128          # dout/din 128-tiles (6)

    @property
    def NT(self):
        return self.SP // 128         # s 128-tiles (9)

    @property
    def NB(self):
        return self.SP // self.SQB    # sq blocks (3)

    @property
    def ROPE_LEN(self):
        return self.ROPE_END - self.ROPE_START


def round_f32r(x: np.ndarray) -> np.ndarray:
    """Round fp32 to the fp32r format (11 mantissa bits, RNE)."""
    b = np.ascontiguousarray(x, dtype=np.float32).view(np.uint32)
    low = b & np.uint32(0xFFF)
    b = b & np.uint32(0xFFFFF000)
    rnd = (low > 0x800) | ((low == 0x800) & (((b >> 12) & 1) != 0))
    b = b + (rnd.astype(np.uint32) << 12)
    return b.view(np.float32)


def to_fp8(x: np.ndarray) -> np.ndarray:
    return np.clip(np.asarray(x, np.float32), -240.0, 240.0).astype(
        ml_dtypes.float8_e4m3)


def fold_pairs(a: np.ndarray) -> np.ndarray:
    """[T*256, C] -> [128, T*2*C]: row r=(2t+i)*128+p lands at [p, (t*2+i)*C+c].
    SBUF tile t is then viewed [128, 2, C] for DoubleRow (contracts 2 K-tiles).
    """
    R, C = a.shape
    T = R // 256
    assert R == T * 256
    out = np.zeros((128, T * 2 * C), a.dtype)
    for t in range(T):
        for i in range(2):
            out[:, (t * 2 + i) * C:(t * 2 + i + 1) * C] = \
                a[(2 * t + i) * 128:(2 * t + i + 1) * 128, :]
    return out


# ---------------- device program ------------------------------------------


def build_nc(cfg: CFG):
    nc = bacc.Bacc(trn_type="TRN2", target_bir_lowering=False, debug=False)
    KT, NT, NB, SQB, SP = cfg.KT, cfg.NT, cfg.NB, cfg.SQB, cfg.SP
    H, HD, D = cfg.H, cfg.HD, cfg.D
    RS, RE, DS, DE = cfg.ROPE_START, cfg.ROPE_END, cfg.DET_START, cfg.DET_END
    RL = cfg.ROPE_LEN
    DET = DE - DS
    NP = 3                              # K-tile pairs covering D=768

    EVQ = 1.0 / (cfg.S_H * cfg.S_W * math.sqrt(HD))   # psum -> q_raw
    EVK = 1.0 / (cfg.S_H * cfg.S_W)                   # psum -> k_raw
    EVV = cfg.S_V / (cfg.S_H * cfg.S_W)               # psum -> v8
    EVO = 1.0 / (cfg.S_C * cfg.S_OW)                  # psum -> out
    EXPS = 1.0 / (cfg.S_Q8 * cfg.S_K8)                # exp input scale
    EXPB = math.log(cfg.S_E)                          # exp bias -> e8 = S_E*exp
    PADP = cfg.S - 8 * 128                            # 105: valid rows, strip 8

    # ---- dram parameters (per core) ----
    d_hs8 = nc.dram_tensor("hs8", [128, NP * 2 * SP], F8, kind="ExternalInput").ap()
    d_qw8 = nc.dram_tensor("qw8", [128, NP * 2 * D], F8, kind="ExternalInput").ap()
    d_kw8 = nc.dram_tensor("kw8", [128, NP * 2 * D], F8, kind="ExternalInput").ap()
    d_vw8 = nc.dram_tensor("vw8", [128, NP * 2 * D], F8, kind="ExternalInput").ap()
    d_ow8 = nc.dram_tensor("ow8", [128, NP * 2 * D], F8, kind="ExternalInput").ap()
    d_g8 = nc.dram_tensor("g8", [64, NT * 2 * SP], F8, kind="ExternalInput").ap()
    d_i8 = nc.dram_tensor("i8", [64, 256], F8, kind="ExternalInput").ap()
    d_z8 = nc.dram_tensor("z8", [128, SP], F8, kind="ExternalInput").ap()
    d_qb = nc.dram_tensor("qb16", [1, D], BF16, kind="ExternalInput").ap()
    d_vb = nc.dram_tensor("vb16", [1, D], BF16, kind="ExternalInput").ap()
    d_ob = nc.dram_tensor("ob16", [1, D], BF16, kind="ExternalInput").ap()
    d_ones = nc.dram_tensor("ones16", [1, 512], BF16, kind="ExternalInput").ap()
    d_cos = nc.dram_tensor("cos16", [128, RL], F32, kind="ExternalInput").ap()
    d_sin = nc.dram_tensor("sin16", [128, RL], F32, kind="ExternalInput").ap()
    d_rotT = nc.dram_tensor("rotT", [128, 128], F32R, kind="ExternalInput").ap()
    d_masks = nc.dram_tensor("masks", [1, 256], F32R, kind="ExternalInput").ap()
    d_ph = nc.dram_tensor("ph", [1, DET], F32, kind="ExternalInput").ap()
    d_pw = nc.dram_tensor("pw", [1, DET], F32, kind="ExternalInput").ap()
    d_out = nc.dram_tensor("out", [SP, D], F32, kind="ExternalOutput").ap()

    with tile.TileContext(nc) as tc, ExitStack() as gctx:
        gsb = gctx.enter_context(tc.tile_pool(name="gsb", bufs=1))

        # ---------------- persistent tiles --------------------------------
        # fp8 score operands: [128, 2, SP]; parity 1 zeroed (DoubleRow K-pad)
        t_q8 = [gsb.tile([128, 2 * SP], F8, tag=f"q8_{m}", name=f"q8_{m}")
                for m in range(KT)]
        t_k8 = [gsb.tile([128, 2 * SP], F8, tag=f"k8_{m}", name=f"k8_{m}")
                for m in range(KT)]
        q8v = [t[:, :].rearrange("p (i c) -> p i c", c=SP) for t in t_q8]
        k8v = [t[:, :].rearrange("p (i c) -> p i c", c=SP) for t in t_k8]
        # v in strip-pair layout [128, 2, H, 65] (65th col = S_V ones)
        t_v8 = [gsb.tile([128, 2 * H * 65], F8, tag=f"v8_{t}", name=f"v8_{t}")
                for t in range(4)]
        t_v8l = gsb.tile([128, H * 65], F8, tag="v8_4", name="v8_4")
        v8v = [t[:, :].rearrange("p (i h j) -> p i h j", i=2, h=H) for t in t_v8]
        v8lv = t_v8l[:, :].rearrange("p (h j) -> p h j", h=H)
        # gate, folded per key strip: [64, 2, SP]
        t_g8 = [gsb.tile([64, 2 * SP], F8, tag=f"g8_{t}", name=f"g8_{t}")
                for t in range(NT)]
        g8v = [t[:, :].rearrange("p (i c) -> p i c", c=SP) for t in t_g8]
        t_i8 = gsb.tile([64, 256], F8, tag="i8")
        i8v = t_i8[:, :].rearrange("p (i c) -> p i c", c=128)
        # raw (pre-rope) projections, f32r
        t_qr = [gsb.tile([128, SP], F32R, tag=f"qr_{m}", name=f"qr_{m}")
                for m in range(KT)]
        t_kr = [gsb.tile([128, SP], F32R, tag=f"kr_{m}", name=f"kr_{m}")
                for m in range(KT)]
        # o-proj weights (fp8 pairs) + small rows
        t_ow8 = [gsb.tile([128, 2 * D], F8, tag=f"ow8_{t}", name=f"ow8_{t}")
                 for t in range(NP)]
        ow8v = [t[:, :].rearrange("p (i c) -> p i c", c=D) for t in t_ow8]
        t_qb = gsb.tile([1, D], BF16, tag="qb16")
        t_vb = gsb.tile([1, D], BF16, tag="vb16")
        t_ob = gsb.tile([1, D], BF16, tag="ob16")
        t_ones = gsb.tile([1, 512], BF16, tag="ones16")
        t_cos = gsb.tile([128, RL], F32, tag="cos16")
        t_sin = gsb.tile([128, RL], F32, tag="sin16")
        t_rotT = gsb.tile([128, 128], F32R, tag="rotT")
        # bc broadcast row: value S_C on row 64 (outer-product lhsT)
        t_scrow = gsb.tile([65, 64], F32R, tag="scrow")
        nc.vector.memset(t_scrow[:], cfg.S_C)
        # exp bias column: ln(S_E) (== ln(S_Q8), used by the tem tables too)
        assert cfg.S_E == cfg.S_Q8
        t_expb = gsb.tile([128, 1], F32, tag="expb")
        nc.vector.memset(t_expb[:], EXPB)

        # ==================== phase 1 + 2: projections & rope ===============
        with ExitStack() as p1:
            sb1 = p1.enter_context(tc.tile_pool(name="sb1", bufs=1))
            tsb = p1.enter_context(tc.tile_pool(name="tsb", bufs=2))
            ps_qk = p1.enter_context(tc.tile_pool(name="ps_qk", bufs=2, space="PSUM"))
            ps_big = p1.enter_context(tc.tile_pool(name="ps_big", bufs=2, space="PSUM"))
            ps_tem = p1.enter_context(tc.tile_pool(name="ps_tem", bufs=1, space="PSUM"))

            # critical-path loads first: hs8 + q weights interleaved
            t_hs8 = []
            for t in range(NP):
                h = sb1.tile([128, 2 * SP], F8, tag=f"hs8_{t}", name=f"hs8_{t}")
                eng = nc.sync if t % 2 == 0 else nc.scalar
                eng.dma_start(h[:], d_hs8[:, t * 2 * SP:(t + 1) * 2 * SP])
                t_hs8.append(h)
            hs8v = [t[:, :].rearrange("p (i c) -> p i c", c=SP) for t in t_hs8]

            def load_w8(dram, nm):
                ts = []
                for t in range(NP):
                    w = sb1.tile([128, 2 * D], F8, tag=f"{nm}_{t}", name=f"{nm}_{t}")
                    eng = nc.scalar if t % 2 == 0 else nc.sync
                    eng.dma_start(w[:], dram[:, t * 2 * D:(t + 1) * 2 * D])
                    ts.append(w)
                return [t[:, :].rearrange("p (i c) -> p i c", c=D) for t in ts]

            qw8v = load_w8(d_qw8, "qw8")
            nc.sync.dma_start(t_qb[:], d_qb[:, :])
            nc.sync.dma_start(t_ones[:], d_ones[:, :])
            nc.scalar.dma_start(t_cos[:], d_cos[:, :])
            nc.scalar.dma_start(t_sin[:], d_sin[:, :])
            nc.sync.dma_start(t_rotT[:], d_rotT[:, :])
            # zero parity-1 of q8/k8 via DMA (DoubleRow zero K-tile)
            for m in range(KT):
                nc.scalar.dma_start(q8v[m][:, 1, :], d_z8[:, :])
                nc.scalar.dma_start(k8v[m][:, 1, :], d_z8[:, :])

            # det-temp pattern tile: [128, DET] = S_Q8 * (mask_h exp(2ph) + ...)
            t_ms = sb1.tile([1, 256], F32R, tag="ms")
            nc.sync.dma_start(t_ms[:], d_masks[:, :])
            t_ph = sb1.tile([1, DET], F32, tag="ph")
            nc.sync.dma_start(t_ph[:], d_ph[:, :])
            t_pw = sb1.tile([1, DET], F32, tag="pw")
            nc.sync.dma_start(t_pw[:], d_pw[:, :])
            t_eh = sb1.tile([1, DET], F32R, tag="eh")
            nc.scalar.activation(t_eh[:], t_ph[:], AF.Exp,
                                 bias=t_expb[0:1, :], scale=cfg.P_SCALE)
            t_ew = sb1.tile([1, DET], F32R, tag="ew")
            nc.scalar.activation(t_ew[:], t_pw[:], AF.Exp,
                                 bias=t_expb[0:1, :], scale=cfg.P_SCALE)
            p_tem = ps_tem.tile([128, DET], F32, tag="tem")
            nc.tensor.matmul(p_tem[:], t_ms[0:1, 0:128], t_eh[:], start=True, stop=False)
            nc.tensor.matmul(p_tem[:], t_ms[0:1, 128:256], t_ew[:], start=False, stop=True)

            def finalize(raw, dst8v):
                """RoPE + det-temp + prefix/tail, raw f32r -> fp8 (x S_Q8)."""
                p_rot = ps_big.tile([128, RL], F32, tag="big", name="p_rot")
                for c0 in range(0, RL, 512):
                    nc.tensor.matmul(p_rot[:, c0:c0 + 512], t_rotT[:],
                                     raw[:, RS + c0:RS + c0 + 512],
                                     start=True, stop=True)
                tmp1 = tsb.tile([128, RL], F32, tag="tmp1", name="tmp1")
                nc.vector.tensor_mul(tmp1[:], p_rot[:], t_sin[:])
                tmp2 = tsb.tile([128, RL], F32, tag="tmp2", name="tmp2")
                nc.gpsimd.tensor_mul(tmp2[:], raw[:, RS:RE], t_cos[:])
                nc.vector.tensor_add(dst8v[:, 0, RS:RE], tmp2[:], tmp1[:])
                nc.vector.tensor_mul(dst8v[:, 0, DS:DE], raw[:, DS:DE], p_tem[:])
                nc.vector.tensor_scalar_mul(dst8v[:, 0, 0:RS], raw[:, 0:RS], cfg.S_Q8)
                nc.vector.tensor_scalar_mul(dst8v[:, 0, DE:SP], raw[:, DE:SP], cfg.S_Q8)

            # ---- q: psum[sq?? no: out rows = dout strip m, cols = s block ----
            for m in range(KT):
                for b in range(NB):
                    b0 = b * SQB
                    p = ps_qk.tile([128, SQB], F32, tag="qp", name="qp")
                    for t in range(NP):
                        nc.tensor.matmul(p[:], qw8v[t][:, :, m * 128:(m + 1) * 128],
                                         hs8v[t][:, :, b0:b0 + SQB],
                                         start=(t == 0), stop=False, perf_mode=DR)
                    nc.tensor.matmul(p[:], t_qb[0:1, m * 128:(m + 1) * 128],
                                     t_ones[0:1, 0:SQB], start=False, stop=True)
                    nc.vector.tensor_scalar_mul(t_qr[m][:, b0:b0 + SQB], p[:], EVQ)
                if m > 0:
                    finalize(t_qr[m - 1], q8v[m - 1])

            # ---- k ----
            kw8v = load_w8(d_kw8, "kw8")
            # gate DMAs: after the k weights, well before attention needs them
            for t in range(NT):
                nc.sync.dma_start(t_g8[t][:], d_g8[:, t * 2 * SP:(t + 1) * 2 * SP])
            nc.sync.dma_start(t_i8[:], d_i8[:, :])
            for m in range(KT):
                for b in range(NB):
                    b0 = b * SQB
                    p = ps_qk.tile([128, SQB], F32, tag="qp", name="kp")
                    for t in range(NP):
                        nc.tensor.matmul(p[:], kw8v[t][:, :, m * 128:(m + 1) * 128],
                                         hs8v[t][:, :, b0:b0 + SQB],
                                         start=(t == 0), stop=(t == NP - 1),
                                         perf_mode=DR)
                    nc.gpsimd.tensor_scalar_mul(t_kr[m][:, b0:b0 + SQB], p[:], EVK)
                if m == 0:
                    finalize(t_qr[KT - 1], q8v[KT - 1])
                if m > 0:
                    finalize(t_kr[m - 1], k8v[m - 1])
            finalize(t_kr[KT - 1], k8v[KT - 1])

            # ---- v: out[s_tile, dout] with bias row; evict to fp8 pairs ----
            vw8v = load_w8(d_vw8, "vw8")
            nc.sync.dma_start(t_vb[:], d_vb[:, :])
            nc.sync.dma_start(t_ob[:], d_ob[:, :])
            for t in range(NP):
                nc.sync.dma_start(t_ow8[t][:], d_ow8[:, t * 2 * D:(t + 1) * 2 * D])
            for mt in range(NT):
                p = ps_big.tile([128, RL], F32, tag="big", name="vp")
                for n0, nw in ((0, 512), (512, 256)):
                    for t in range(NP):
                        nc.tensor.matmul(p[:, n0:n0 + nw],
                                         hs8v[t][:, :, mt * 128:(mt + 1) * 128],
                                         vw8v[t][:, :, n0:n0 + nw],
                                         start=(t == 0), stop=False, perf_mode=DR)
                    nc.tensor.matmul(p[:, n0:n0 + nw], t_ones[0:1, 0:128],
                                     t_vb[0:1, n0:n0 + nw], start=False, stop=True)
                pv = p[:, 0:D].rearrange("p (h j) -> p h j", j=HD)
                if mt < 8:
                    nc.vector.tensor_scalar_mul(v8v[mt // 2][:, mt % 2, :, 0:HD],
                                                pv, EVV)
                else:
                    nc.vector.tensor_scalar_mul(v8lv[:, :, 0:HD], pv, EVV)
            # ones columns (denominator) + pad-row zeroing
            for t in range(4):
                nc.vector.memset(v8v[t][:, :, :, HD:65], cfg.S_V)
            nc.vector.memset(v8lv[PADP:128, :, :], 0.0)
            nc.vector.memset(v8lv[0:PADP, :, HD:65], cfg.S_V)

        # ==================== phase 3: attention + output projection ========
        with ExitStack() as p3:
            esb = p3.enter_context(tc.tile_pool(name="esb", bufs=4))
            csb = p3.enter_context(tc.tile_pool(name="csb", bufs=2))
            ps_sc = p3.enter_context(tc.tile_pool(name="ps_sc", bufs=2, space="PSUM"))
            ps_ctx = p3.enter_context(tc.tile_pool(name="ps_ctx", bufs=4, space="PSUM"))

            for b in range(NB):
                b0 = b * SQB
                t_c8 = [csb.tile([128, 2 * SQB], F8, tag=f"c8_{t}", name=f"c8_{t}")
                        for t in range(NP)]
                c8v = [t[:, :].rearrange("p (i c) -> p i c", c=SQB) for t in t_c8]

                def norm_head(h, pc):
                    t_rc = csb.tile([65, SQB], F32R, tag="rc", bufs=3, name="rc")
                    with nc.allow_low_precision(reason="f32r recip of softmax denom"):
                        nc.vector.reciprocal(t_rc[64:65, :], pc[64:65, :])
                    p_bc = ps_ctx.tile([64, SQB], F32, tag="ctx", name="bc")
                    nc.tensor.matmul(p_bc[:], t_scrow[64:65, :], t_rc[64:65, :],
                                     start=True, stop=True)
                    t_cu = csb.tile([64, SQB], F32, tag="cu", bufs=3, name="cu")
                    nc.gpsimd.tensor_copy(t_cu[:], pc[0:64, :])
                    kt = h // 2
                    hr = (h % 2) * 64
                    nc.vector.tensor_mul(c8v[kt // 2][hr:hr + 64, kt % 2, :],
                                         t_cu[:], p_bc[:])

                for hp in range(H // 2):
                    h0, h1 = 2 * hp, 2 * hp + 1
                    p_ctx0 = ps_ctx.tile([65, SQB], F32, tag="ctx", name="c0")
                    p_ctx1 = ps_ctx.tile([65, SQB], F32, tag="ctx", name="c1")
                    e_pair = None
                    for skt in range(NT):
                        tp, par = skt // 2, skt % 2
                        sc = ps_sc.tile([128, 1024], F32, tag="sc", name="sc")
                        g = g8v[skt][:, :, b0:b0 + SQB]
                        nc.tensor.matmul(sc[:, 0:SQB], i8v, g,
                                         start=True, stop=False, perf_mode=DR)
                        nc.tensor.matmul(sc[:, 512:512 + SQB], i8v, g,
                                         start=True, stop=False, perf_mode=DR)
                        nc.tensor.matmul(sc[:, 0:SQB],
                                         k8v[hp][0:64, :, skt * 128:(skt + 1) * 128],
                                         q8v[hp][0:64, :, b0:b0 + SQB],
                                         start=False, stop=True, perf_mode=DR)
                        nc.tensor.matmul(sc[:, 512:512 + SQB],
                                         k8v[hp][64:128, :, skt * 128:(skt + 1) * 128],
                                         q8v[hp][64:128, :, b0:b0 + SQB],
                                         start=False, stop=True, perf_mode=DR)
                        src = sc[:, :].rearrange("p (i c) -> p i c", c=512)[:, :, 0:SQB]
                        if skt < 8:
                            if par == 0:
                                e_pair = esb.tile([128, 4 * SQB], F8, tag="e", name="e")
                            e4 = e_pair[:, :].rearrange("p (s h c) -> p s h c",
                                                        s=2, h=2)
                            nc.scalar.activation(e4[:, par, :, :], src, AF.Exp,
                                                 bias=t_expb[:], scale=EXPS)
                            if par == 1:
                                for hh, pc in ((0, p_ctx0), (1, p_ctx1)):
                                    nc.tensor.matmul(
                                        pc[:], v8v[tp][:, :, 2 * hp + hh, :],
                                        e4[:, :, hh, :],
                                        start=(tp == 0), stop=False, perf_mode=DR)
                        else:
                            e_last = esb.tile([128, 2 * SQB], F8, tag="el", name="el")
                            e2 = e_last[:, :].rearrange("p (h c) -> p h c", h=2)
                            nc.scalar.activation(e2[:, :, :], src, AF.Exp,
                                                 bias=t_expb[:], scale=EXPS)
                            for hh, pc in ((0, p_ctx0), (1, p_ctx1)):
                                nc.tensor.matmul(pc[:], v8lv[:, 2 * hp + hh, :],
                                                 e2[:, hh, :],
                                                 start=False, stop=True)
                    norm_head(h0, p_ctx0)
                    norm_head(h1, p_ctx1)

                # output projection for this sq block
                for mt in range(SQB // 128):
                    t_out = csb.tile([128, D], F32, tag="out", name="t_out")
                    r0 = b0 + mt * 128
                    for n0, nw in ((0, 512), (512, 256)):
                        p_o = ps_ctx.tile([128, nw], F32, tag="ctx", name="po")
                        for t in range(NP):
                            nc.tensor.matmul(p_o[:],
                                             c8v[t][:, :, mt * 128:(mt + 1) * 128],
                                             ow8v[t][:, :, n0:n0 + nw],
                                             start=(t == 0), stop=False, perf_mode=DR)
                        nc.tensor.matmul(p_o[:], t_ones[0:1, 0:128],
                                         t_ob[0:1, n0:n0 + nw], start=False, stop=True)
                        nc.vector.tensor_scalar_mul(t_out[:, n0:n0 + nw], p_o[:], EVO)
                        nc.sync.dma_start(d_out[r0:r0 + 128, n0:n0 + nw],
                                          t_out[:, n0:n0 + nw])

    nc.compile()
    return nc


# ---------------- host-side prep + dispatch --------------------------------


def _host_prep(cfg: CFG, hidden_states, q_w, q_b, k_w, v_w, v_b, o_w, o_b,
               cos, sin, ph, pw, gate):
    SP, H, HD, D, S = cfg.SP, cfg.H, cfg.HD, cfg.D, cfg.S
    DET = cfg.DET_END - cfg.DET_START
    half = HD // 2

    shared = {}
    shared["qw8"] = fold_pairs(to_fp8(q_w.T * cfg.S_W))
    shared["kw8"] = fold_pairs(to_fp8(k_w.T * cfg.S_W))
    shared["vw8"] = fold_pairs(to_fp8(v_w.T * cfg.S_W))
    shared["ow8"] = fold_pairs(to_fp8(o_w.T * cfg.S_OW))

    # gate, transposed to [sk, sq], scaled by S_Q8*S_K8, folded per strip
    gateT = np.zeros((SP, SP), np.float32)
    gateT[:S, :S] = gate[0, 0].T
    g8 = np.zeros((64, cfg.NT * 2 * SP), np.float32)
    for skt in range(cfg.NT):
        blk = gateT[skt * 128:(skt + 1) * 128, :] * (cfg.S_Q8 * cfg.S_K8)
        g8[:, (2 * skt) * SP:(2 * skt + 1) * SP] = blk[0:64]
        g8[:, (2 * skt + 1) * SP:(2 * skt + 2) * SP] = blk[64:128]
    shared["g8"] = to_fp8(g8)

    # folded identity for the gate DoubleRow copy
    i8 = np.zeros((64, 256), np.float32)
    for p in range(64):
        i8[p, p] = 1.0
        i8[p, 128 + 64 + p] = 1.0
    shared["i8"] = to_fp8(i8)
    shared["z8"] = to_fp8(np.zeros((128, SP), np.float32))

    shared["qb16"] = (q_b * (cfg.S_H * cfg.S_W)).reshape(1, D).astype(
        ml_dtypes.bfloat16)
    shared["vb16"] = (v_b * (cfg.S_H * cfg.S_W)).reshape(1, D).astype(
        ml_dtypes.bfloat16)
    shared["ob16"] = (o_b * (cfg.S_C * cfg.S_OW)).reshape(1, D).astype(
        ml_dtypes.bfloat16)
    shared["ones16"] = np.ones((1, 512), np.float32).astype(ml_dtypes.bfloat16)

    # rope tables: [128, RL] = two stacked head-copies, pre-scaled by S_Q8
    cosT = cos.T.astype(np.float32) * cfg.S_Q8
    sinT = sin.T.astype(np.float32) * cfg.S_Q8
    shared["cos16"] = np.vstack([cosT, cosT]).astype(np.float32)
    shared["sin16"] = np.vstack([sinT, sinT]).astype(np.float32)

    R = np.zeros((128, 128), np.float32)
    for blk in range(2):
        o = blk * HD
        for j in range(half):
            R[o + j, o + half + j] = -1.0
            R[o + half + j, o + j] = 1.0
    shared["rotT"] = round_f32r(R.T)

    maska = np.zeros((1, 128), np.float32)
    maskb = np.zeros((1, 128), np.float32)
    for p in range(128):
        if (p % HD) < half:
            maska[0, p] = 1.0
        else:
            maskb[0, p] = 1.0
    shared["masks"] = round_f32r(np.concatenate([maska, maskb], axis=1))
    shared["ph"] = ph.astype(np.float32).reshape(1, DET)
    shared["pw"] = pw.astype(np.float32).reshape(1, DET)

    in_maps = []
    for c in range(cfg.N_CORES):
        hsT = np.zeros((D, SP), np.float32)
        hsT[:, :S] = hidden_states[c].T
        m = dict(shared)
        m["hs8"] = fold_pairs(to_fp8(hsT * cfg.S_H))
        in_maps.append(m)
    return in_maps


_NC_CACHE = {}


def kernel(hidden_states, q_w, q_b, k_w, v_w, v_b, o_w, o_b,
           cos, sin, ph, pw, gate,
           rope_start=5, rope_end=1029, det_start=1029, det_end=1129):
    cfg = CFG()
    in_maps = _host_prep(cfg, np.asarray(hidden_states, np.float32),
                         np.asarray(q_w, np.float32), np.asarray(q_b, np.float32),
                         np.asarray(k_w, np.float32), np.asarray(v_w, np.float32),
                         np.asarray(v_b, np.float32), np.asarray(o_w, np.float32),
                         np.asarray(o_b, np.float32), np.asarray(cos, np.float32),
                         np.asarray(sin, np.float32), np.asarray(ph, np.float32),
                         np.asarray(pw, np.float32), np.asarray(gate, np.float32))
    if "nc" not in _NC_CACHE:
        _NC_CACHE["nc"] = build_nc(cfg)
    nc = _NC_CACHE["nc"]
    res = run_bass_kernel_spmd(nc, in_maps, list(range(cfg.N_CORES)))
    out = np.stack([res.results[c]["out"][:cfg.S] for c in range(cfg.N_CORES)])
    return out.astype(np.float32)
